# revision 16
# baseline (speedup 1.0000x reference)
"""Trainium2 Bass kernel for the ActorCriticCriterion (AIC) masked REINFORCE loss.

Reference computation (per the oracle):
    at_or_after_eos = cumsum(seq == 0, axis=1) > 0
    seq_z  = where(at_or_after_eos, 0, seq)
    mask   = concat([ones(B,1), (seq_z > 0)[:, :-1]], axis=1)
    loss   = sum(-logp * (reward - value) * mask) / sum(mask)

Key identity used here: with eos = index of first zero in the row (or T-1 if
none — the two cases give identical masks), mask[t] = (t <= eos), and
sum(mask) per row = eos + 1.  So per 128-row tile we need:
    rmax = max_t((seq[t] == 0) * (1023 - t))      # = 1023 - eos (0 if no zero)
    eos  = 1023 - rmax
    mask = (iota <= eos)              (+ fused per-row accumulation of den)
    num += sum(logp * (value - reward) * mask)    (fused tensor_tensor_reduce)

Sharding: pure data-parallel over B; each of the 8 cores processes 1024 rows.
Each core emits per-partition partial sums [128, k]; the host sums the 8
cores' partials and divides.
"""

import os
import numpy as np

B, T = 8192, 1024
NCORES = 8
ROWS = B // NCORES          # rows per core
P = 128                     # SBUF partitions
A = 2                       # row-groups of 128 per block (tile = [128, A, T])

_CACHE: dict = {}


def _build_program(rows: int, d_engine: str = "pool", cand_engine: str = "dve",
                   q_engine: str = "pool"):
    """Build the Bass/Tile program for one core processing `rows` rows."""
    from contextlib import ExitStack

    import concourse.bacc as bacc
    import concourse.mybir as mybir
    import concourse.tile as tile

    nblk = rows // (P * A)
    assert nblk * P * A == rows

    f32 = mybir.dt.float32
    i32 = mybir.dt.int32
    Op = mybir.AluOpType

    # Bacc (not raw Bass): its compile pipeline splits multi-sem sync waits
    # into event-semaphore instructions — this walrus build allows at most
    # one wait per engine instruction.
    nc = bacc.Bacc()
    seq = nc.dram_tensor("seq", [rows, T], i32, kind="ExternalInput")
    lp = nc.dram_tensor("lp", [rows, T], f32, kind="ExternalInput")
    val = nc.dram_tensor("val", [rows, T], f32, kind="ExternalInput")
    rew = nc.dram_tensor("rew", [rows, T], f32, kind="ExternalInput")
    out_num = nc.dram_tensor("out_num", [P, nblk], f32, kind="ExternalOutput")
    out_den = nc.dram_tensor("out_den", [P, nblk * A], f32, kind="ExternalOutput")

    def dram_block(t, i):
        # rows [i*A*P, (i+1)*A*P) as [p, a, t] with row = a*P + p
        return t[i * A * P:(i + 1) * A * P, :].rearrange("(a p) t -> p a t", p=P)

    use_stt = bool(int(os.environ.get("K_STT", "1")))
    use_ts_accum = bool(int(os.environ.get("K_TS_ACCUM", "1")))

    with ExitStack() as ctx:
        tc = ctx.enter_context(tile.TileContext(nc))
        const_pool = ctx.enter_context(tc.tile_pool(name="const", bufs=1))
        in_pool = ctx.enter_context(tc.tile_pool(name="in", bufs=3))
        scr_pool = ctx.enter_context(tc.tile_pool(name="scr", bufs=2))
        acc_pool = ctx.enter_context(tc.tile_pool(name="acc", bufs=1))

        # Constant index tiles: iota[t] = t, riota[t] = 1023 - t (per
        # row-group), generated once on gpsimd.
        iota = const_pool.tile([P, A, T], i32)
        riota = const_pool.tile([P, A, T], i32)
        nc.gpsimd.iota(iota[:], pattern=[[0, A], [1, T]], base=0,
                       channel_multiplier=0)
        nc.gpsimd.iota(riota[:], pattern=[[0, A], [-1, T]], base=T - 1,
                       channel_multiplier=0)
        num_acc = acc_pool.tile([P, nblk], f32)
        den_acc = acc_pool.tile([P, nblk * A], f32)

        for i in range(nblk):
            seq_t = in_pool.tile([P, A, T], i32, tag="seq")
            lp_t = in_pool.tile([P, A, T], f32, tag="lp")
            val_t = in_pool.tile([P, A, T], f32, tag="val")
            rew_t = in_pool.tile([P, A, T], f32, tag="rew")
            nc.sync.dma_start(out=seq_t[:], in_=dram_block(seq, i))
            nc.sync.dma_start(out=lp_t[:], in_=dram_block(lp, i))
            nc.sync.dma_start(out=val_t[:], in_=dram_block(val, i))
            nc.sync.dma_start(out=rew_t[:], in_=dram_block(rew, i))

            # cand = (seq == 0) * (1023 - t); rmax[p,a] = max_t cand.
            # eos = first-zero index (or 1023 if the row has no zero).
            cand = scr_pool.tile([P, A, T], f32, tag="cand")
            if use_stt:
                nc.vector.scalar_tensor_tensor(
                    out=cand[:], in0=seq_t[:], scalar=0.0, in1=riota[:],
                    op0=Op.is_equal, op1=Op.mult)
            else:
                eq01 = scr_pool.tile([P, A, T], f32, tag="eq01")
                nc.vector.tensor_scalar(
                    out=eq01[:], in0=seq_t[:], scalar1=0.0, scalar2=None,
                    op0=Op.is_equal)
                nc.vector.tensor_tensor(
                    out=cand[:], in0=eq01[:], in1=riota[:], op=Op.mult)

            rmax = scr_pool.tile([P, A], f32, tag="rmax")
            if use_ts_accum:
                junk = scr_pool.tile([P, A, T], f32, tag="junk")
                for a in range(A):
                    nc.vector.tensor_scalar(
                        out=junk[:, a, :], in0=cand[:, a, :], scalar1=1.0,
                        scalar2=None, op0=Op.mult, op1=Op.max,
                        accum_out=rmax[:, a:a + 1])
            else:
                nc.vector.tensor_reduce(
                    out=rmax[:], in_=cand[:], axis=mybir.AxisListType.X,
                    op=Op.max)

            # eos = 1023 - rmax ; den partial per row = eos + 1 = 1024 - rmax
            eos = scr_pool.tile([P, A], f32, tag="eos")
            nc.vector.tensor_scalar(
                out=eos[:], in0=rmax[:], scalar1=-1.0, scalar2=float(T - 1),
                op0=Op.mult, op1=Op.add)
            nc.vector.tensor_scalar(
                out=den_acc[:, i * A:(i + 1) * A], in0=rmax[:], scalar1=-1.0,
                scalar2=float(T), op0=Op.mult, op1=Op.add)

            # mask[p, a, t] = (t <= eos[p, a])
            mask = scr_pool.tile([P, A, T], f32, tag="mask")
            for a in range(A):
                nc.vector.tensor_scalar(
                    out=mask[:, a, :], in0=iota[:, a, :],
                    scalar1=eos[:, a:a + 1], scalar2=None, op0=Op.is_le)

            # d = value - reward (pool); q = logp * d (pool)
            d = scr_pool.tile([P, A, T], f32, tag="d")
            eng = nc.gpsimd if d_engine == "pool" else nc.vector
            eng.tensor_tensor(out=d[:], in0=val_t[:], in1=rew_t[:],
                              op=Op.subtract)
            q = scr_pool.tile([P, A, T], f32, tag="q")
            eng = nc.gpsimd if q_engine == "pool" else nc.vector
            eng.tensor_tensor(out=q[:], in0=lp_t[:], in1=d[:], op=Op.mult)

            # num partial: sum(q * mask)
            mq = scr_pool.tile([P, A, T], f32, tag="cand")
            nc.vector.tensor_tensor(
                out=mq[:], in0=q[:], in1=mask[:], op=Op.mult)
            if use_ts_accum:
                junk2 = scr_pool.tile([P, A, T], f32, tag="junk")
                nc.vector.tensor_scalar(
                    out=junk2[:], in0=mq[:], scalar1=1.0,
                    scalar2=None, op0=Op.mult, op1=Op.add,
                    accum_out=num_acc[:, i:i + 1])
            else:
                nc.vector.tensor_reduce(
                    out=num_acc[:, i:i + 1],
                    in_=mq[:].rearrange("p a t -> p (a t)"),
                    axis=mybir.AxisListType.X, op=Op.add)

        nc.sync.dma_start(out=out_num[:], in_=num_acc[:])
        nc.sync.dma_start(out=out_den[:], in_=den_acc[:])

    nc.finalize()
    return nc


def kernel(sample_seq, sample_seqLogprobs, sample_value, sample_reward):
    from concourse.bass_utils import run_bass_kernel_spmd

    seq = np.ascontiguousarray(np.asarray(sample_seq, dtype=np.int32))
    lp = np.ascontiguousarray(np.asarray(sample_seqLogprobs, dtype=np.float32))
    val = np.ascontiguousarray(np.asarray(sample_value, dtype=np.float32))
    rew = np.ascontiguousarray(np.asarray(sample_reward, dtype=np.float32))
    assert seq.shape == (B, T)

    if "nc" not in _CACHE:
        _CACHE["nc"] = _build_program(
            ROWS,
            d_engine=os.environ.get("K_D_ENGINE", "pool"),
            cand_engine=os.environ.get("K_CAND_ENGINE", "dve"),
            q_engine=os.environ.get("K_Q_ENGINE", "pool"),
        )
    nc = _CACHE["nc"]

    in_maps = []
    for c in range(NCORES):
        sl = slice(c * ROWS, (c + 1) * ROWS)
        in_maps.append({
            "seq": seq[sl], "lp": lp[sl], "val": val[sl], "rew": rew[sl],
        })

    trace = bool(int(os.environ.get("K_TRACE", "0")))
    res = run_bass_kernel_spmd(nc, in_maps, core_ids=list(range(NCORES)),
                               trace=trace)
    if trace:
        _CACHE["exec_time_ns"] = res.exec_time_ns
        _CACHE["trace"] = res.instructions_and_trace
    num = 0.0
    den = 0.0
    for r in res.results:
        num += float(np.asarray(r["out_num"], dtype=np.float64).sum())
        den += float(np.asarray(r["out_den"], dtype=np.float64).sum())
    return np.float32(num / den)


# revision 19
# speedup vs baseline: 1.1405x; 1.1405x over previous
"""Trainium2 Bass kernel for the ActorCriticCriterion (AIC) masked REINFORCE loss.

Reference computation (per the oracle):
    at_or_after_eos = cumsum(seq == 0, axis=1) > 0
    seq_z  = where(at_or_after_eos, 0, seq)
    mask   = concat([ones(B,1), (seq_z > 0)[:, :-1]], axis=1)
    loss   = sum(-logp * (reward - value) * mask) / sum(mask)

Identity used: mask[t] = AND(seq[0..t-1] != 0) with mask[0] = 1 — computed
directly with one DVE tensor_tensor_scan (op0=logical_and) per 128-row group,
writing to a shifted access pattern (the leading ones column is a memset).

Per [128, A, T] block:
    pool:  d = value - reward                       (gpsimd tensor_tensor)
    DVE:   mask via logical_and scan (shifted)
           q  = logp * d
           mq = q * mask
    PE:    ones[128,1].T @ mq  chunks -> PSUM num[1, A*T]  (accum over blocks)
           ones[128,1].T @ mask chunks -> PSUM den[1, A*T]
Outputs are the two [1, A*T] PSUM accumulators; the host sums them and
divides.  Sharding: pure data-parallel over B across 8 cores (1024 rows each).
"""

import os
import numpy as np

B, T = 8192, 1024
NCORES = 8
ROWS = B // NCORES          # rows per core
P = 128                     # SBUF partitions
A = 2                       # row-groups of 128 per block (tile = [128, A, T])
MMCHUNK = 512               # matmul free-dim chunk (one PSUM bank)

_CACHE: dict = {}


def _build_program(rows: int, d_engine: str = "pool"):
    """Build the Bass/Tile program for one core processing `rows` rows."""
    from contextlib import ExitStack

    import concourse.bacc as bacc
    import concourse.mybir as mybir
    import concourse.tile as tile

    nblk = rows // (P * A)
    assert nblk * P * A == rows

    f32 = mybir.dt.float32
    i32 = mybir.dt.int32
    Op = mybir.AluOpType

    # Bacc (not raw Bass): its compile pipeline splits multi-sem sync waits
    # into event-semaphore instructions — this walrus build allows at most
    # one wait per engine instruction.
    nc = bacc.Bacc()
    seq = nc.dram_tensor("seq", [rows, T], i32, kind="ExternalInput")
    lp = nc.dram_tensor("lp", [rows, T], f32, kind="ExternalInput")
    val = nc.dram_tensor("val", [rows, T], f32, kind="ExternalInput")
    rew = nc.dram_tensor("rew", [rows, T], f32, kind="ExternalInput")
    out_num = nc.dram_tensor("out_num", [1, A * T], f32, kind="ExternalOutput")
    out_den = nc.dram_tensor("out_den", [1, A * T], f32, kind="ExternalOutput")

    def dram_block(t, i):
        # rows [i*A*P, (i+1)*A*P) as [p, a, t] with row = a*P + p
        return t[i * A * P:(i + 1) * A * P, :].rearrange("(a p) t -> p a t", p=P)

    with ExitStack() as ctx:
        tc = ctx.enter_context(tile.TileContext(nc))
        const_pool = ctx.enter_context(tc.tile_pool(name="const", bufs=1))
        in_pool = ctx.enter_context(tc.tile_pool(name="in", bufs=3))
        scr_pool = ctx.enter_context(tc.tile_pool(name="scr", bufs=2))
        psum_pool = ctx.enter_context(
            tc.tile_pool(name="psum", bufs=1, space="PSUM"))

        ones = const_pool.tile([P, 1], f32)
        nc.vector.memset(ones[:], 1.0)

        num_ps = psum_pool.tile([1, A * T], f32)
        den_ps = psum_pool.tile([1, A * T], f32)

        for i in range(nblk):
            seq_t = in_pool.tile([P, A, T], i32, tag="seq")
            lp_t = in_pool.tile([P, A, T], f32, tag="lp")
            val_t = in_pool.tile([P, A, T], f32, tag="val")
            rew_t = in_pool.tile([P, A, T], f32, tag="rew")
            nc.sync.dma_start(out=seq_t[:], in_=dram_block(seq, i))
            nc.sync.dma_start(out=lp_t[:], in_=dram_block(lp, i))
            nc.sync.dma_start(out=val_t[:], in_=dram_block(val, i))
            nc.sync.dma_start(out=rew_t[:], in_=dram_block(rew, i))

            # mask[p,a,0] = 1; mask[p,a,t] = AND(seq[p,a,0..t-1] != 0)
            mask = scr_pool.tile([P, A, T], f32, tag="mask")
            nc.vector.memset(mask[:, :, 0:1], 1.0)
            for a in range(A):
                nc.vector.tensor_tensor_scan(
                    out=mask[:, a, 1:T], data0=seq_t[:, a, 0:T - 1],
                    data1=seq_t[:, a, 0:T - 1], initial=1.0,
                    op0=Op.logical_and, op1=Op.bypass)

            # d = value - reward
            d = scr_pool.tile([P, A, T], f32, tag="d")
            eng = nc.gpsimd if d_engine == "pool" else nc.vector
            eng.tensor_tensor(out=d[:], in0=val_t[:], in1=rew_t[:],
                              op=Op.subtract)

            # q = logp * d ; mq = q * mask
            q = scr_pool.tile([P, A, T], f32, tag="q")
            nc.vector.tensor_tensor(out=q[:], in0=lp_t[:], in1=d[:], op=Op.mult)
            mq = scr_pool.tile([P, A, T], f32, tag="mq")
            nc.vector.tensor_tensor(out=mq[:], in0=q[:], in1=mask[:],
                                    op=Op.mult)

            # PE column sums (contract over the 128 partitions), accumulated
            # across blocks in PSUM.
            for a in range(A):
                for c in range(0, T, MMCHUNK):
                    sl = slice(a * T + c, a * T + c + MMCHUNK)
                    nc.tensor.matmul(
                        out=num_ps[:, sl], lhsT=ones[:],
                        rhs=mq[:, a, c:c + MMCHUNK],
                        start=(i == 0), stop=(i == nblk - 1))
                    nc.tensor.matmul(
                        out=den_ps[:, sl], lhsT=ones[:],
                        rhs=mask[:, a, c:c + MMCHUNK],
                        start=(i == 0), stop=(i == nblk - 1))

        # PSUM can't be DMA'd directly — bounce through SBUF on the (idle)
        # scalar engine, which sits closest to PSUM.
        num_sb = const_pool.tile([1, A * T], f32)
        den_sb = const_pool.tile([1, A * T], f32)
        nc.scalar.copy(num_sb[:], num_ps[:])
        nc.scalar.copy(den_sb[:], den_ps[:])
        nc.sync.dma_start(out=out_num[:], in_=num_sb[:])
        nc.sync.dma_start(out=out_den[:], in_=den_sb[:])

    nc.finalize()
    return nc


def kernel(sample_seq, sample_seqLogprobs, sample_value, sample_reward):
    from concourse.bass_utils import run_bass_kernel_spmd

    seq = np.ascontiguousarray(np.asarray(sample_seq, dtype=np.int32))
    lp = np.ascontiguousarray(np.asarray(sample_seqLogprobs, dtype=np.float32))
    val = np.ascontiguousarray(np.asarray(sample_value, dtype=np.float32))
    rew = np.ascontiguousarray(np.asarray(sample_reward, dtype=np.float32))
    assert seq.shape == (B, T)

    if "nc" not in _CACHE:
        _CACHE["nc"] = _build_program(
            ROWS, d_engine=os.environ.get("K_D_ENGINE", "pool"))
    nc = _CACHE["nc"]

    in_maps = []
    for c in range(NCORES):
        sl = slice(c * ROWS, (c + 1) * ROWS)
        in_maps.append({
            "seq": seq[sl], "lp": lp[sl], "val": val[sl], "rew": rew[sl],
        })

    trace = bool(int(os.environ.get("K_TRACE", "0")))
    res = run_bass_kernel_spmd(nc, in_maps, core_ids=list(range(NCORES)),
                               trace=trace)
    if trace:
        _CACHE["exec_time_ns"] = res.exec_time_ns
        _CACHE["trace"] = res.instructions_and_trace
    num = 0.0
    den = 0.0
    for r in res.results:
        num += float(np.asarray(r["out_num"], dtype=np.float64).sum())
        den += float(np.asarray(r["out_den"], dtype=np.float64).sum())
    return np.float32(num / den)
